# revision 9
# baseline (speedup 1.0000x reference)
"""Trainium2 Bass kernel for GQA attention (32 q heads / 16 kv heads, head_dim
128, L=2048, D=4608) with RoPE, tanh softcap 50, causal mask, o_proj.

Strategy: tensor-parallel over heads across 8 NeuronCores. Core c computes
q-heads 4c..4c+3 and kv-heads 2c..2c+1 end-to-end (QKV projections, RoPE,
softcapped causal attention, and the partial o_proj against its 512 columns of
wo); the host sums the 8 partial [L, D] outputs.

v1 changes over the baseline (613us measured):
  - all DRAM inputs are pre-blocked on the host into SBUF-image layout so each
    weight / x-chunk loads with a few large contiguous dma_starts instead of 36
    small ones (the sync queue serialized ~150 descriptors at ~616ns each,
    stretching the startup bubble to 39us).
  - phase 2 is software-pipelined at instruction granularity: score tiles of
    attention group i interleave with PV/normalize/transpose work of group i-1
    and o_proj chunks as PE filler, keeping the tensor engine busy while the
    scalar engine chews tanh+exp (previously PE idled, causing HAM
    re-throttles to half clock).
  - exp is batched over pairs of score tiles ([128,1024] activations) to
    amortize the scalar engine's 352-cycle pipeline fill.
  - all PSUM->SBUF drains moved from DVE to the idle Pool engine (nc.gpsimd).
  - o_proj output staged and stored as bf16 (host sums partials in f32).
  - x chunks processed in order [3,0,1,2]; the V projection of the last
    processed chunk (2) is deferred into the phase-2 pipeline as PE filler for
    the ACT-bound attention startup.
"""

from collections import deque

import numpy as np
import ml_dtypes

import concourse.bass as bass
import concourse.mybir as mybir
import concourse.tile as tile
from concourse.masks import make_identity
from concourse import bacc

F32 = mybir.dt.float32
BF16 = mybir.dt.bfloat16
BF16_NP = ml_dtypes.bfloat16
AF = mybir.ActivationFunctionType

N_HEADS = 32
N_KV = 16
HEAD_DIM = 128
ROPE_THETA = 10000.0
SOFTCAP = 50.0
SCALE = 1.0 / 12.0  # 1/sqrt(144)
L = 2048
D = 4608
N_CORES = 8
QH = N_HEADS // N_CORES        # 4 local q heads
KVH = N_KV // N_CORES          # 2 local kv heads
KC = D // 128                  # 36 contraction chunks
NQ = L // 512                  # 4 l-chunks of 512
LT = L // 128                  # 16 l-tiles of 128
QUART = 9 * 512                # x/wq quarter width (9 k-chunks)
CHUNK_ORDER = [3, 0, 1, 2]     # phase-1 processing order; V of chunk 2 deferred


def _emit(nc):
    # DRAM tensors in SBUF-image layout (see make_in_maps)
    xq_d = nc.dram_tensor("xq", [NQ, 4, 128, QUART], BF16, kind="ExternalInput")
    wqq_d = nc.dram_tensor("wqq", [4, 128, QUART], BF16, kind="ExternalInput")
    wkh_d = nc.dram_tensor("wkh", [2, 128, 18 * 256], BF16, kind="ExternalInput")
    wvh_d = nc.dram_tensor("wvh", [2, 128, 18 * 256], BF16, kind="ExternalInput")
    wo4_d = nc.dram_tensor("wo4", [QH, 128, D], BF16, kind="ExternalInput")
    cost_d = nc.dram_tensor("cost", [128, L], BF16, kind="ExternalInput")
    sint_d = nc.dram_tensor("sint", [128, L], BF16, kind="ExternalInput")
    masks_d = nc.dram_tensor("masks", [128, 4 * 512], BF16, kind="ExternalInput")
    out_d = nc.dram_tensor("out", [L, D], BF16, kind="ExternalOutput")

    with tile.TileContext(nc) as tc:
        with (
            tc.tile_pool(name="const", bufs=1) as const,
            tc.tile_pool(name="persist", bufs=1) as persist,
            tc.tile_pool(name="xpb", bufs=1) as xpb,    # x chunks 0 and 2; outlives phase 1
            tc.tile_pool(name="wvp", bufs=1) as wvp,    # wv weights; outlives phase 1
        ):
            QT = [persist.tile([128, L], BF16, tag=f"qt{h}", name=f"qt{h}") for h in range(QH)]
            KT = [persist.tile([128, L], BF16, tag=f"kt{g}", name=f"kt{g}") for g in range(KVH)]
            # V extended with a ones column per k-tile: [128, 16*129]
            VE = [persist.tile([128, LT * 129], BF16, tag=f"ve{g}", name=f"ve{g}") for g in range(KVH)]

            ident = const.tile([128, 128], BF16)
            cost = const.tile([128, L], BF16)
            sint = const.tile([128, L], BF16)
            maskt = const.tile([128, 4 * 512], BF16)
            wvs = wvp.tile([128, KC * 256], BF16, name="wvs")

            deferred = _phase1(nc, tc, ident, cost, sint, maskt, wvs, QT, KT, VE,
                               xq_d, wqq_d, wkh_d, wvh_d, cost_d, sint_d, masks_d, xpb)
            _phase2(nc, tc, ident, maskt, QT, KT, VE, wvs, wo4_d, out_d, deferred)
    return nc


def _phase1(nc, tc, ident, cost, sint, maskt, wvs, QT, KT, VE,
            xq_d, wqq_d, wkh_d, wvh_d, cost_d, sint_d, masks_d, xpb):
    with (
        tc.tile_pool(name="xpa", bufs=1) as xpa,
        tc.tile_pool(name="wqk", bufs=1) as wqk,
        tc.tile_pool(name="rtmp", bufs=3) as rtmp,
        tc.tile_pool(name="pj_psum", bufs=2, space="PSUM") as pj_psum,
        tc.tile_pool(name="vp_psum", bufs=2, space="PSUM") as vp_psum,
    ):
        wqs = wqk.tile([128, KC * 512], BF16, name="wqs")
        wks = wqk.tile([128, KC * 256], BF16, name="wks")

        def load_x(nq, dst):
            for j in range(4):
                nc.sync.dma_start(dst[:, j * QUART:(j + 1) * QUART], xq_d[nq, j])

        # first chunk processed is CHUNK_ORDER[0]; interleave its x quarters
        # with wq quarters so the first Q matmuls start within ~8us
        x_first = xpa.tile([128, KC * 512], BF16, tag="xa", name="xa")
        for j in range(4):
            nc.sync.dma_start(wqs[:, j * QUART:(j + 1) * QUART], wqq_d[j])
            nc.sync.dma_start(x_first[:, j * QUART:(j + 1) * QUART],
                              xq_d[CHUNK_ORDER[0], j])
        nc.sync.dma_start(cost[:], cost_d[:])
        nc.sync.dma_start(sint[:], sint_d[:])
        for j in range(2):
            nc.sync.dma_start(wks[:, j * 18 * 256:(j + 1) * 18 * 256], wkh_d[j])
        for j in range(2):
            nc.sync.dma_start(wvs[:, j * 18 * 256:(j + 1) * 18 * 256], wvh_d[j])
        nc.sync.dma_start(maskt[:], masks_d[:])
        make_identity(nc, ident[:])

        def drain_rope(ps, dst, nq):
            """psum [128,512] f32 -> rope -> dst bf16 [128,512] slice."""
            cols = slice(nq * 512, (nq + 1) * 512)
            raw = rtmp.tile([128, 512], F32, tag="raw")
            nc.scalar.activation(raw[:], ps[:], AF.Copy)
            swap = rtmp.tile([128, 512], F32, tag="swap")
            nc.scalar.activation(swap[0:64, :], ps[64:128, :], AF.Copy)
            nc.scalar.activation(swap[64:128, :], ps[0:64, :], AF.Copy)
            nc.vector.tensor_mul(raw[:], raw[:], cost[:, cols])
            nc.vector.tensor_mul(swap[:], swap[:], sint[:, cols])
            nc.vector.tensor_add(dst[:, cols], raw[:], swap[:])

        def alloc_x(i):
            pool = xpb if i % 2 == 1 else xpa
            tag = "xb" if i % 2 == 1 else "xa"
            return pool.tile([128, KC * 512], BF16, tag=tag, name=tag)

        xc = x_first
        for i, nq in enumerate(CHUNK_ORDER):
            if i + 1 < len(CHUNK_ORDER):
                # prefetch next chunk; single-buffered tags alternate between
                # the xpa and xpb pools, so this write only WARs against the
                # chunk processed two iterations ago
                xnext = alloc_x(i + 1)
                load_x(CHUNK_ORDER[i + 1], xnext)
            else:
                xnext = None
            for h in range(QH):
                ps = pj_psum.tile([128, 512], F32, tag="qk")
                for k in range(KC):
                    nc.tensor.matmul(
                        ps[:], wqs[:, k * 512 + h * 128:k * 512 + (h + 1) * 128],
                        xc[:, k * 512:(k + 1) * 512],
                        start=(k == 0), stop=(k == KC - 1))
                drain_rope(ps, QT[h], nq)
            for g in range(KVH):
                ps = pj_psum.tile([128, 512], F32, tag="qk")
                for k in range(KC):
                    nc.tensor.matmul(
                        ps[:], wks[:, k * 256 + g * 128:k * 256 + (g + 1) * 128],
                        xc[:, k * 512:(k + 1) * 512],
                        start=(k == 0), stop=(k == KC - 1))
                drain_rope(ps, KT[g], nq)
            if i == len(CHUNK_ORDER) - 1:
                return [(nq, sub, xc) for sub in range(4)]  # deferred V units
            for sub in range(4):
                mk = nq * 4 + sub
                ps = vp_psum.tile([128, KVH * 128], F32, tag="vps")
                for k in range(KC):
                    nc.tensor.matmul(
                        ps[:], xc[:, k * 512 + sub * 128:k * 512 + (sub + 1) * 128],
                        wvs[:, k * 256:(k + 1) * 256],
                        start=(k == 0), stop=(k == KC - 1))
                for g in range(KVH):
                    nc.vector.tensor_copy(
                        VE[g][:, mk * 129:mk * 129 + 128],
                        ps[:, g * 128:(g + 1) * 128])
                    nc.gpsimd.memset(VE[g][:, mk * 129 + 128:mk * 129 + 129], 1.0)
            xc = xnext


def _phase2(nc, tc, ident, maskt, QT, KT, VE, wvs, wo4_d, out_d, deferred_v):
    with (
        tc.tile_pool(name="wo", bufs=1) as wop,
        tc.tile_pool(name="pt", bufs=2) as ptp,
        tc.tile_pool(name="tt", bufs=2) as ttp,
        tc.tile_pool(name="attnt", bufs=2) as attp,
        tc.tile_pool(name="small", bufs=2) as small,
        tc.tile_pool(name="ostage", bufs=2) as ostage,
        tc.tile_pool(name="sc_psum", bufs=2, space="PSUM") as sc_psum,
        tc.tile_pool(name="pv_psum", bufs=2, space="PSUM") as pv_psum,
        tc.tile_pool(name="op_psum", bufs=2, space="PSUM") as op_psum,
        tc.tile_pool(name="atrv_psum", bufs=1, space="PSUM") as atrv_psum,
    ):
        WO = wop.tile([128, QH * D], BF16, name="wos")
        for h in range(QH):
            nc.sync.dma_start(WO[:, h * D:(h + 1) * D], wo4_d[h])

        groups = [(nq, h) for nq in range(NQ) for h in range(QH)]
        state = {}
        att_of = {nq: [None] * QH for nq in range(NQ)}

        # filler queue: (group_idx, pe_cost_ns, emit_fn)
        filler = deque()

        def make_v_unit(vnq, sub, xc):
            def emit():
                mk = vnq * 4 + sub
                ps = atrv_psum.tile([128, KVH * 128], F32, tag="vps")
                for k in range(KC):
                    nc.tensor.matmul(
                        ps[:], xc[:, k * 512 + sub * 128:k * 512 + (sub + 1) * 128],
                        wvs[:, k * 256:(k + 1) * 256],
                        start=(k == 0), stop=(k == KC - 1))
                for g in range(KVH):
                    nc.vector.tensor_copy(
                        VE[g][:, mk * 129:mk * 129 + 128],
                        ps[:, g * 128:(g + 1) * 128])
                    nc.gpsimd.memset(VE[g][:, mk * 129 + 128:mk * 129 + 129], 1.0)
            return (1, 3900, emit)

        for (vnq, sub, xc) in deferred_v:
            filler.append(make_v_unit(vnq, sub, xc))

        def pump(ns):
            while ns > 0 and filler:
                _, cost, fn = filler.popleft()
                fn()
                ns -= cost

        def pump_guard(idx):
            # bound pipeline lag: everything enqueued at group <= idx-2 must
            # be emitted before group idx starts reusing its tile buffers
            while filler and filler[0][0] <= idx - 2:
                filler.popleft()[2]()

        def emit_scores_pair(nq, h, pk, pts):
            g = h // 2
            tt = ttp.tile([128, 1024], F32, tag="tt")
            for half in range(2):
                mk = 2 * pk + half
                o = mk - 4 * nq
                c0 = max(0, o) * 128
                w = 512 - c0
                ps = sc_psum.tile([128, 512], F32, tag="sc")
                nc.tensor.matmul(
                    ps[:, 0:w], KT[g][:, mk * 128:(mk + 1) * 128],
                    QT[h][:, nq * 512 + c0:(nq + 1) * 512])
                nc.scalar.activation(
                    tt[:, half * 512 + c0:half * 512 + 512], ps[:, 0:w],
                    AF.Tanh, scale=SCALE / SOFTCAP)
            pt = ptp.tile([128, 1024], BF16, tag=f"pt{pk}")
            nc.scalar.activation(pt[:], tt[:], AF.Exp, scale=SOFTCAP)
            for half in range(2):
                mk = 2 * pk + half
                o = mk - 4 * nq
                if o >= 0:
                    c0 = o * 128
                    sl = slice(half * 512 + c0, half * 512 + 512)
                    nc.gpsimd.tensor_mul(
                        pt[:, sl], pt[:, sl], maskt[:, o * 512 + c0:(o + 1) * 512])
            pts.append(pt)

        def make_pv_unit(idx, nq, h, s):
            def emit():
                st = state[(nq, h)]
                g = h // 2
                nks = 4 * nq + s + 1
                pv = pv_psum.tile([128, 129], F32, tag="pv")
                for mk in range(nks):
                    pk, half = divmod(mk, 2)
                    nc.tensor.matmul(
                        pv[:],
                        st["pts"][pk][:, half * 512 + s * 128:half * 512 + (s + 1) * 128],
                        VE[g][:, mk * 129:(mk + 1) * 129],
                        start=(mk == 0), stop=(mk == nks - 1))
                recip = small.tile([128, 1], F32, tag="recip")
                nc.vector.reciprocal(recip[:], pv[:, 128:129])
                aq = small.tile([128, 128], BF16, tag=f"attnq{s}")
                nc.vector.tensor_scalar_mul(aq[:], pv[:, 0:128], recip[:])
                st["attnq"][s] = aq
            return (idx, 200 + (4 * nq + s + 1) * 95, emit)

        def make_tr_unit(idx, nq, h, s):
            def emit():
                st = state[(nq, h)]
                tp = atrv_psum.tile([128, 128], BF16, tag="atr")
                nc.tensor.transpose(tp[:], st["attnq"][s][:], ident[:])
                nc.vector.tensor_copy(st["attnT"][:, s * 128:(s + 1) * 128], tp[:])
            return (idx, 180, emit)

        def make_oproj_unit(idx, nq, s, jg, ob):
            def emit():
                for j in range(3 * jg, 3 * jg + 3):
                    po = op_psum.tile([128, 512], F32, tag="op")
                    for h in range(QH):
                        nc.tensor.matmul(
                            po[:], att_of[nq][h][:, s * 128:(s + 1) * 128],
                            WO[:, h * D + j * 512:h * D + (j + 1) * 512],
                            start=(h == 0), stop=(h == QH - 1))
                    nc.vector.tensor_copy(ob[:, (j - 3 * jg) * 512:(j - 3 * jg + 1) * 512], po[:])
                row = nq * 512 + s * 128
                nc.sync.dma_start(
                    out_d[row:row + 128, jg * 1536:(jg + 1) * 1536], ob[:])
            return (idx, 2700, emit)

        for i, (nq, h) in enumerate(groups):
            nkt = 4 * nq + 4
            attnT = attp.tile([128, 512], BF16, tag=f"at{h}", name=f"at{h}")
            att_of[nq][h] = attnT
            st = {"pts": [], "attnq": [None] * 4, "attnT": attnT}
            state[(nq, h)] = st
            for pk in range(nkt // 2):
                pump_guard(i)
                emit_scores_pair(nq, h, pk, st["pts"])
                pump(2400)
            for s in range(4):
                filler.append(make_pv_unit(i, nq, h, s))
                filler.append(make_tr_unit(i, nq, h, s))
            if h == QH - 1:
                for s in range(4):
                    for jg in range(3):
                        ob = ostage.tile([128, 1536], BF16, tag="ob")
                        filler.append(make_oproj_unit(i, nq, s, jg, ob))
        while filler:
            filler.popleft()[2]()


_CACHED_NC = {}


def build():
    if "nc" not in _CACHED_NC:
        nc = bacc.Bacc("TRN2", target_bir_lowering=False, debug=False)
        _emit(nc)
        nc.compile()
        _CACHED_NC["nc"] = nc
    return _CACHED_NC["nc"]


def host_tables():
    inv_freq = 1.0 / (ROPE_THETA ** (np.arange(0, HEAD_DIM, 2, dtype=np.float32) / HEAD_DIM))
    ang = np.arange(L, dtype=np.float32)[:, None] * inv_freq[None, :]  # [L, 64]
    cos, sin = np.cos(ang), np.sin(ang)
    cosT = np.concatenate([cos.T, cos.T], axis=0).astype(BF16_NP)
    sinT = np.concatenate([-sin.T, sin.T], axis=0).astype(BF16_NP)
    return np.ascontiguousarray(cosT), np.ascontiguousarray(sinT)


def host_masks():
    k = np.arange(128)[:, None]
    q = np.arange(512)[None, :]
    m = np.concatenate([(q >= k + 128 * o) for o in range(4)], axis=1).astype(BF16_NP)
    return np.ascontiguousarray(m)  # [128, 2048]


def make_in_maps(x, wq, wk, wv, wo):
    cosT, sinT = host_tables()
    masks = host_masks()
    xt = x.reshape(L, D).T.astype(BF16_NP)  # [D, L]
    # x image: [nq, j, p, k9*512+c] = xt[(9j+k9)*128+p, nq*512+c]
    xi = xt.reshape(KC, 128, NQ, 512).transpose(2, 0, 1, 3)  # [NQ, KC, 128, 512]
    xi = xi.reshape(NQ, 4, 9, 128, 512).transpose(0, 1, 3, 2, 4)
    xi = np.ascontiguousarray(xi.reshape(NQ, 4, 128, QUART))
    in_maps = []
    for c in range(N_CORES):
        qs = slice(c * QH * 128, (c + 1) * QH * 128)
        kvs = slice(c * KVH * 128, (c + 1) * KVH * 128)
        wqt = wq[qs].T.astype(BF16_NP)   # [D, 512]
        wkt = wk[kvs].T.astype(BF16_NP)  # [D, 256]
        wvt = wv[kvs].T.astype(BF16_NP)
        wot = wo[:, qs].T.astype(BF16_NP)  # [512, D]
        wqi = wqt.reshape(KC, 128, 512).transpose(1, 0, 2)
        wqi = np.ascontiguousarray(
            wqi.reshape(128, 4, 9, 512).transpose(1, 0, 2, 3).reshape(4, 128, QUART))
        wki = wkt.reshape(KC, 128, 256).transpose(1, 0, 2)
        wki = np.ascontiguousarray(
            wki.reshape(128, 2, 18, 256).transpose(1, 0, 2, 3).reshape(2, 128, 18 * 256))
        wvi = wvt.reshape(KC, 128, 256).transpose(1, 0, 2)
        wvi = np.ascontiguousarray(
            wvi.reshape(128, 2, 18, 256).transpose(1, 0, 2, 3).reshape(2, 128, 18 * 256))
        wo4 = np.ascontiguousarray(wot.reshape(QH, 128, D))
        in_maps.append({
            "xq": xi,
            "wqq": wqi,
            "wkh": wki,
            "wvh": wvi,
            "wo4": wo4,
            "cost": cosT,
            "sint": sinT,
            "masks": masks,
        })
    return in_maps


def run(inputs, trace=False, trace_kwargs=None):
    from concourse.bass_utils import run_bass_kernel_spmd

    nc = build()
    x = np.asarray(inputs["x"], dtype=np.float32)
    in_maps = make_in_maps(
        x,
        np.asarray(inputs["wq"], dtype=np.float32),
        np.asarray(inputs["wk"], dtype=np.float32),
        np.asarray(inputs["wv"], dtype=np.float32),
        np.asarray(inputs["wo"], dtype=np.float32),
    )
    res = run_bass_kernel_spmd(
        nc, in_maps, core_ids=list(range(N_CORES)),
        trace=trace, **(trace_kwargs or {}))
    out = np.zeros((L, D), dtype=np.float32)
    for c in range(N_CORES):
        out += res.results[c]["out"].astype(np.float32)
    return out.reshape(x.shape), res


def kernel(**inputs) -> np.ndarray:
    out, _ = run(inputs, trace=False)
    return out


# revision 12
# speedup vs baseline: 1.1487x; 1.1487x over previous
"""Trainium2 Bass kernel for GQA attention (32 q heads / 16 kv heads, head_dim
128, L=2048, D=4608) with RoPE, tanh softcap 50, causal mask, o_proj.

Strategy: tensor-parallel over heads across 8 NeuronCores. Core c computes
q-heads 4c..4c+3 and kv-heads 2c..2c+1 end-to-end (QKV projections, RoPE,
softcapped causal attention, and the partial o_proj against its 512 columns of
wo); the host sums the 8 partial [L, D] outputs.

v2 design notes (baseline 613us, v1 675us-at-2.0GHz):
  - all DRAM inputs are pre-blocked on the host into SBUF-image layout so each
    weight / x-chunk loads with a few large contiguous dma_starts instead of 36
    small ones (the sync queue serialized ~150 descriptors at ~620ns each,
    which stretched the startup bubble to 39us).
  - score matmuls write fp16 into PSUM, two 512-wide tiles per bank, so tanh
    runs once per [128,1024] pair and exp once per [128,2048] 2-pair chunk.
    This cuts the scalar engine's per-op ~480-cycle pipeline-fill overhead,
    which had made ACT the phase-2 bottleneck (291us busy in v1). tanh values
    are <=0.2 (softcap 50, |scores|<=~10) so fp16 intermediates lose nothing.
  - phase 2 is software-pipelined at instruction granularity: score pairs of
    attention group i interleave with PV/normalize/transpose units of group
    i-1 and o_proj chunk units as PE filler, so the tensor engine never idles
    long enough for the HAM clock gate to re-throttle it.
  - PSUM->SBUF drains on DVE (GPSIMD/Pool cannot access PSUM); the SBUF-only
    causal-mask multiplies run on the otherwise-idle Pool engine.
  - o_proj psum pool is triple-buffered; output staged and stored as bf16
    (host sums the 8 partials in f32).
"""

from collections import deque

import numpy as np
import ml_dtypes

import concourse.bass as bass
import concourse.mybir as mybir
import concourse.tile as tile
from concourse.masks import make_identity
from concourse import bacc

F32 = mybir.dt.float32
FP16 = mybir.dt.float16
BF16 = mybir.dt.bfloat16
BF16_NP = ml_dtypes.bfloat16
AF = mybir.ActivationFunctionType

N_HEADS = 32
N_KV = 16
HEAD_DIM = 128
ROPE_THETA = 10000.0
SOFTCAP = 50.0
SCALE = 1.0 / 12.0  # 1/sqrt(144)
L = 2048
D = 4608
N_CORES = 8
QH = N_HEADS // N_CORES        # 4 local q heads
KVH = N_KV // N_CORES          # 2 local kv heads
KC = D // 128                  # 36 contraction chunks
NQ = L // 512                  # 4 l-chunks of 512
LT = L // 128                  # 16 l-tiles of 128
QUART = 9 * 512                # x/wq quarter width (9 k-chunks)


def _emit(nc):
    # DRAM tensors in SBUF-image layout (see make_in_maps)
    xq_d = nc.dram_tensor("xq", [NQ, 4, 128, QUART], BF16, kind="ExternalInput")
    wqq_d = nc.dram_tensor("wqq", [4, 128, QUART], BF16, kind="ExternalInput")
    wkh_d = nc.dram_tensor("wkh", [2, 128, 18 * 256], BF16, kind="ExternalInput")
    wvh_d = nc.dram_tensor("wvh", [2, 128, 18 * 256], BF16, kind="ExternalInput")
    wo4_d = nc.dram_tensor("wo4", [QH, 128, D], BF16, kind="ExternalInput")
    cost_d = nc.dram_tensor("cost", [128, L], BF16, kind="ExternalInput")
    sint_d = nc.dram_tensor("sint", [128, L], BF16, kind="ExternalInput")
    masks_d = nc.dram_tensor("masks", [128, 4 * 512], BF16, kind="ExternalInput")
    out_d = nc.dram_tensor("out", [L, D], BF16, kind="ExternalOutput")

    with tile.TileContext(nc) as tc:
        with (
            tc.tile_pool(name="const", bufs=1) as const,
            tc.tile_pool(name="persist", bufs=1) as persist,
        ):
            QT = [persist.tile([128, L], BF16, tag=f"qt{h}", name=f"qt{h}") for h in range(QH)]
            KT = [persist.tile([128, L], BF16, tag=f"kt{g}", name=f"kt{g}") for g in range(KVH)]
            # V extended with a ones column per k-tile: [128, 16*129]
            VE = [persist.tile([128, LT * 129], BF16, tag=f"ve{g}", name=f"ve{g}") for g in range(KVH)]

            ident = const.tile([128, 128], BF16)
            cost = const.tile([128, L], BF16)
            sint = const.tile([128, L], BF16)
            maskt = const.tile([128, 4 * 512], BF16)

            _phase1(nc, tc, ident, cost, sint, maskt, QT, KT, VE,
                    xq_d, wqq_d, wkh_d, wvh_d, cost_d, sint_d, masks_d)
            _phase2(nc, tc, ident, maskt, QT, KT, VE, wo4_d, out_d)
    return nc


def _phase1(nc, tc, ident, cost, sint, maskt, QT, KT, VE,
            xq_d, wqq_d, wkh_d, wvh_d, cost_d, sint_d, masks_d):
    with (
        tc.tile_pool(name="xcol", bufs=2) as xcol,
        tc.tile_pool(name="wts", bufs=1) as wts,
        tc.tile_pool(name="rtmp", bufs=3) as rtmp,
        tc.tile_pool(name="pj_psum", bufs=2, space="PSUM") as pj_psum,
        tc.tile_pool(name="vp_psum", bufs=2, space="PSUM") as vp_psum,
    ):
        wqs = wts.tile([128, KC * 512], BF16, name="wqs")
        wks = wts.tile([128, KC * 256], BF16, name="wks")
        wvs = wts.tile([128, KC * 256], BF16, name="wvs")

        def load_x(nq, dst):
            for j in range(4):
                nc.sync.dma_start(dst[:, j * QUART:(j + 1) * QUART], xq_d[nq, j])

        # interleave wq and x(0) quarters so the first Q matmuls start early
        x_first = xcol.tile([128, KC * 512], BF16, tag="x", name="x0")
        for j in range(4):
            nc.sync.dma_start(wqs[:, j * QUART:(j + 1) * QUART], wqq_d[j])
            nc.sync.dma_start(x_first[:, j * QUART:(j + 1) * QUART], xq_d[0, j])
        nc.sync.dma_start(cost[:], cost_d[:])
        nc.sync.dma_start(sint[:], sint_d[:])
        for j in range(2):
            nc.sync.dma_start(wks[:, j * 18 * 256:(j + 1) * 18 * 256], wkh_d[j])
        for j in range(2):
            nc.sync.dma_start(wvs[:, j * 18 * 256:(j + 1) * 18 * 256], wvh_d[j])
        nc.sync.dma_start(maskt[:], masks_d[:])
        make_identity(nc, ident[:])

        def drain_rope(ps, dst, nq):
            """psum [128,512] f32 -> rope -> dst bf16 [128,512] slice."""
            cols = slice(nq * 512, (nq + 1) * 512)
            raw = rtmp.tile([128, 512], F32, tag="raw")
            nc.scalar.activation(raw[:], ps[:], AF.Copy)
            swap = rtmp.tile([128, 512], F32, tag="swap")
            nc.scalar.activation(swap[0:64, :], ps[64:128, :], AF.Copy)
            nc.scalar.activation(swap[64:128, :], ps[0:64, :], AF.Copy)
            nc.vector.tensor_mul(raw[:], raw[:], cost[:, cols])
            nc.vector.tensor_mul(swap[:], swap[:], sint[:, cols])
            nc.vector.tensor_add(dst[:, cols], raw[:], swap[:])

        xc = x_first
        for nq in range(NQ):
            if nq + 1 < NQ:
                xnext = xcol.tile([128, KC * 512], BF16, tag="x", name=f"x{nq+1}")
                load_x(nq + 1, xnext)
            for h in range(QH):
                ps = pj_psum.tile([128, 512], F32, tag="qk")
                for k in range(KC):
                    nc.tensor.matmul(
                        ps[:], wqs[:, k * 512 + h * 128:k * 512 + (h + 1) * 128],
                        xc[:, k * 512:(k + 1) * 512],
                        start=(k == 0), stop=(k == KC - 1))
                drain_rope(ps, QT[h], nq)
            for g in range(KVH):
                ps = pj_psum.tile([128, 512], F32, tag="qk")
                for k in range(KC):
                    nc.tensor.matmul(
                        ps[:], wks[:, k * 256 + g * 128:k * 256 + (g + 1) * 128],
                        xc[:, k * 512:(k + 1) * 512],
                        start=(k == 0), stop=(k == KC - 1))
                drain_rope(ps, KT[g], nq)
            for sub in range(4):
                mk = nq * 4 + sub
                ps = vp_psum.tile([128, KVH * 128], F32, tag="vps")
                for k in range(KC):
                    nc.tensor.matmul(
                        ps[:], xc[:, k * 512 + sub * 128:k * 512 + (sub + 1) * 128],
                        wvs[:, k * 256:(k + 1) * 256],
                        start=(k == 0), stop=(k == KC - 1))
                for g in range(KVH):
                    nc.vector.tensor_copy(
                        VE[g][:, mk * 129:mk * 129 + 128],
                        ps[:, g * 128:(g + 1) * 128])
                    nc.gpsimd.memset(VE[g][:, mk * 129 + 128:mk * 129 + 129], 1.0)
            if nq + 1 < NQ:
                xc = xnext


def _phase2(nc, tc, ident, maskt, QT, KT, VE, wo4_d, out_d):
    with (
        tc.tile_pool(name="wo", bufs=1) as wop,
        tc.tile_pool(name="pt", bufs=2) as ptp,
        tc.tile_pool(name="tt", bufs=2) as ttp,
        tc.tile_pool(name="attnt", bufs=2) as attp,
        tc.tile_pool(name="small", bufs=2) as small,
        tc.tile_pool(name="ostage", bufs=2) as ostage,
        tc.tile_pool(name="sc_psum", bufs=2, space="PSUM") as sc_psum,
        tc.tile_pool(name="pv_psum", bufs=2, space="PSUM") as pv_psum,
        tc.tile_pool(name="op_psum", bufs=3, space="PSUM") as op_psum,
        tc.tile_pool(name="atr_psum", bufs=1, space="PSUM") as atr_psum,
    ):
        WO = wop.tile([128, QH * D], BF16, name="wos")
        for h in range(QH):
            nc.sync.dma_start(WO[:, h * D:(h + 1) * D], wo4_d[h])

        groups = [(nq, h) for nq in range(NQ) for h in range(QH)]
        state = {}
        att_of = {nq: [None] * QH for nq in range(NQ)}

        # filler queue: (group_idx, pe_cost_ns, emit_fn)
        filler = deque()

        def pump(ns):
            while ns > 0 and filler:
                _, cost, fn = filler.popleft()
                fn()
                ns -= cost

        def pump_guard(idx):
            # bound pipeline lag: everything enqueued at group <= idx-2 must
            # be emitted before group idx starts reusing its tile buffers
            while filler and filler[0][0] <= idx - 2:
                filler.popleft()[2]()

        def emit_scores_pair(nq, h, pk, tt, pt):
            """two score tiles -> f32 psum each -> trimmed tanh into the fp16
            tt quad staging; exp runs once per completed [128,2048] quad."""
            g = h // 2
            for half in range(2):
                mk = 2 * pk + half
                o = mk - 4 * nq
                c0 = max(0, o) * 128
                slot = (mk % 4) * 512
                ps = sc_psum.tile([128, 512], F32, tag="sc")
                nc.tensor.matmul(
                    ps[:, c0:512],
                    KT[g][:, mk * 128:(mk + 1) * 128],
                    QT[h][:, nq * 512 + c0:(nq + 1) * 512])
                nc.scalar.activation(
                    tt[:, slot + c0:slot + 512], ps[:, c0:512],
                    AF.Tanh, scale=SCALE / SOFTCAP)
                if mk % 4 == 3:
                    # exp over the completed [128,2048] quad
                    c = mk // 4
                    nc.scalar.activation(
                        pt[:, c * 2048:(c + 1) * 2048], tt[:], AF.Exp,
                        scale=SOFTCAP)

        def emit_masks(nq, h, pt):
            for o in range(4):
                mk = 4 * nq + o
                c0 = o * 128
                sl = slice(mk * 512 + c0, mk * 512 + 512)
                nc.gpsimd.tensor_mul(
                    pt[:, sl], pt[:, sl], maskt[:, o * 512 + c0:(o + 1) * 512])

        def make_pv_unit(idx, nq, h, s):
            def emit():
                st = state[(nq, h)]
                g = h // 2
                nks = 4 * nq + s + 1
                pv = pv_psum.tile([128, 129], F32, tag="pv")
                for mk in range(nks):
                    nc.tensor.matmul(
                        pv[:],
                        st["pt"][:, mk * 512 + s * 128:mk * 512 + (s + 1) * 128],
                        VE[g][:, mk * 129:(mk + 1) * 129],
                        start=(mk == 0), stop=(mk == nks - 1))
                recip = small.tile([128, 1], F32, tag="recip")
                nc.vector.reciprocal(recip[:], pv[:, 128:129])
                aq = small.tile([128, 128], BF16, tag=f"attnq{s}")
                nc.vector.tensor_scalar_mul(aq[:], pv[:, 0:128], recip[:])
                st["attnq"][s] = aq
            return (idx, 200 + (4 * nq + s + 1) * 110, emit)

        def make_tr_unit(idx, nq, h, s):
            def emit():
                st = state[(nq, h)]
                tp = atr_psum.tile([128, 128], BF16, tag="atr")
                nc.tensor.transpose(tp[:], st["attnq"][s][:], ident[:])
                nc.vector.tensor_copy(st["attnT"][:, s * 128:(s + 1) * 128], tp[:])
            return (idx, 180, emit)

        def make_oproj_unit(idx, nq, s, jg, ob):
            def emit():
                for j in range(3 * jg, 3 * jg + 3):
                    po = op_psum.tile([128, 512], F32, tag="op")
                    for h in range(QH):
                        nc.tensor.matmul(
                            po[:], att_of[nq][h][:, s * 128:(s + 1) * 128],
                            WO[:, h * D + j * 512:h * D + (j + 1) * 512],
                            start=(h == 0), stop=(h == QH - 1))
                    nc.vector.tensor_copy(
                        ob[:, (j - 3 * jg) * 512:(j - 3 * jg + 1) * 512], po[:])
                row = nq * 512 + s * 128
                nc.sync.dma_start(
                    out_d[row:row + 128, jg * 1536:(jg + 1) * 1536], ob[:])
            return (idx, 3100, emit)

        for i, (nq, h) in enumerate(groups):
            npairs = 2 * nq + 2
            attnT = attp.tile([128, 512], BF16, tag=f"at{h}", name=f"at{h}")
            att_of[nq][h] = attnT
            pt = ptp.tile([128, LT * 512], BF16, tag="pt", name="pt")
            st = {"pt": pt, "attnq": [None] * 4, "attnT": attnT}
            state[(nq, h)] = st
            tt = None
            for pk in range(npairs):
                pump_guard(i)
                if pk % 2 == 0:
                    tt = ttp.tile([128, 2048], FP16, tag="tt")
                emit_scores_pair(nq, h, pk, tt, pt)
                pump(2200)
            emit_masks(nq, h, pt)
            for s in range(4):
                filler.append(make_pv_unit(i, nq, h, s))
                filler.append(make_tr_unit(i, nq, h, s))
            if h == QH - 1:
                for s in range(4):
                    for jg in range(3):
                        ob = ostage.tile([128, 1536], BF16, tag="ob")
                        filler.append(make_oproj_unit(i, nq, s, jg, ob))
        while filler:
            filler.popleft()[2]()


_CACHED_NC = {}


def build():
    if "nc" not in _CACHED_NC:
        nc = bacc.Bacc("TRN2", target_bir_lowering=False, debug=False)
        _emit(nc)
        nc.compile()
        _CACHED_NC["nc"] = nc
    return _CACHED_NC["nc"]


def host_tables():
    inv_freq = 1.0 / (ROPE_THETA ** (np.arange(0, HEAD_DIM, 2, dtype=np.float32) / HEAD_DIM))
    ang = np.arange(L, dtype=np.float32)[:, None] * inv_freq[None, :]  # [L, 64]
    cos, sin = np.cos(ang), np.sin(ang)
    cosT = np.concatenate([cos.T, cos.T], axis=0).astype(BF16_NP)
    sinT = np.concatenate([-sin.T, sin.T], axis=0).astype(BF16_NP)
    return np.ascontiguousarray(cosT), np.ascontiguousarray(sinT)


def host_masks():
    k = np.arange(128)[:, None]
    q = np.arange(512)[None, :]
    m = np.concatenate([(q >= k + 128 * o) for o in range(4)], axis=1).astype(BF16_NP)
    return np.ascontiguousarray(m)  # [128, 2048]


def make_in_maps(x, wq, wk, wv, wo):
    cosT, sinT = host_tables()
    masks = host_masks()
    xt = x.reshape(L, D).T.astype(BF16_NP)  # [D, L]
    # x image: [nq, j, p, k9*512+c] = xt[(9j+k9)*128+p, nq*512+c]
    xi = xt.reshape(KC, 128, NQ, 512).transpose(2, 0, 1, 3)  # [NQ, KC, 128, 512]
    xi = xi.reshape(NQ, 4, 9, 128, 512).transpose(0, 1, 3, 2, 4)
    xi = np.ascontiguousarray(xi.reshape(NQ, 4, 128, QUART))
    in_maps = []
    for c in range(N_CORES):
        qs = slice(c * QH * 128, (c + 1) * QH * 128)
        kvs = slice(c * KVH * 128, (c + 1) * KVH * 128)
        wqt = wq[qs].T.astype(BF16_NP)   # [D, 512]
        wkt = wk[kvs].T.astype(BF16_NP)  # [D, 256]
        wvt = wv[kvs].T.astype(BF16_NP)
        wot = wo[:, qs].T.astype(BF16_NP)  # [512, D]
        wqi = wqt.reshape(KC, 128, 512).transpose(1, 0, 2)
        wqi = np.ascontiguousarray(
            wqi.reshape(128, 4, 9, 512).transpose(1, 0, 2, 3).reshape(4, 128, QUART))
        wki = wkt.reshape(KC, 128, 256).transpose(1, 0, 2)
        wki = np.ascontiguousarray(
            wki.reshape(128, 2, 18, 256).transpose(1, 0, 2, 3).reshape(2, 128, 18 * 256))
        wvi = wvt.reshape(KC, 128, 256).transpose(1, 0, 2)
        wvi = np.ascontiguousarray(
            wvi.reshape(128, 2, 18, 256).transpose(1, 0, 2, 3).reshape(2, 128, 18 * 256))
        wo4 = np.ascontiguousarray(wot.reshape(QH, 128, D))
        in_maps.append({
            "xq": xi,
            "wqq": wqi,
            "wkh": wki,
            "wvh": wvi,
            "wo4": wo4,
            "cost": cosT,
            "sint": sinT,
            "masks": masks,
        })
    return in_maps


def run(inputs, trace=False, trace_kwargs=None):
    from concourse.bass_utils import run_bass_kernel_spmd

    nc = build()
    x = np.asarray(inputs["x"], dtype=np.float32)
    in_maps = make_in_maps(
        x,
        np.asarray(inputs["wq"], dtype=np.float32),
        np.asarray(inputs["wk"], dtype=np.float32),
        np.asarray(inputs["wv"], dtype=np.float32),
        np.asarray(inputs["wo"], dtype=np.float32),
    )
    res = run_bass_kernel_spmd(
        nc, in_maps, core_ids=list(range(N_CORES)),
        trace=trace, **(trace_kwargs or {}))
    out = np.zeros((L, D), dtype=np.float32)
    for c in range(N_CORES):
        out += res.results[c]["out"].astype(np.float32)
    return out.reshape(x.shape), res


def kernel(**inputs) -> np.ndarray:
    out, _ = run(inputs, trace=False)
    return out


# revision 13
# speedup vs baseline: 1.1645x; 1.0138x over previous
"""Trainium2 Bass kernel for GQA attention (32 q heads / 16 kv heads, head_dim
128, L=2048, D=4608) with RoPE, tanh softcap 50, causal mask, o_proj.

Strategy: tensor-parallel over heads across 8 NeuronCores. Core c computes
q-heads 4c..4c+3 and kv-heads 2c..2c+1 end-to-end (QKV projections, RoPE,
softcapped causal attention, and the partial o_proj against its 512 columns of
wo); the host sums the 8 partial [L, D] outputs.

v3 design notes (baseline 613us, v2b 587us):
  - all DRAM inputs are pre-blocked on the host into SBUF-image layout so each
    weight / x-chunk loads with a few large contiguous dma_starts instead of 36
    small ones (the baseline's sync queue serialized ~150 descriptors at
    ~620ns each, stretching the startup bubble to 39us).
  - softcap pipeline: per-tile tanh (f32 psum -> fp16 staging, causally
    trimmed), then one exp per [128,2048] quad. tanh values are <=0.2
    (softcap 50, |scores|<=~10) so fp16 staging loses nothing; the batching
    amortizes the scalar engine's ~480-cycle per-op pipeline fill.
  - phase 2 is software-pipelined at instruction granularity with two filler
    queues: exp-dependent PV/normalize/transpose units only become eligible
    two score-pairs into the NEXT group (v2b stalled 1.5-3us twice per group
    on PV LDWEIGHTS waiting for exp, re-throttling the HAM clock gate), while
    exp-independent o_proj units fill the early-group windows.
  - V projection of x-chunk 3 is deferred into phase 2 as initial PE filler
    (its psum rides the o_proj pool tag; its x tile and wv weights live in
    pools that outlive phase 1).
  - PSUM->SBUF drains on DVE (GPSIMD/Pool cannot access PSUM); the SBUF-only
    causal-mask multiplies run on the otherwise-idle Pool engine.
  - o_proj psum pool is triple-buffered; output staged and stored as bf16
    (host sums the 8 partials in f32).
"""

from collections import deque

import numpy as np
import ml_dtypes

import concourse.bass as bass
import concourse.mybir as mybir
import concourse.tile as tile
from concourse.masks import make_identity
from concourse import bacc

F32 = mybir.dt.float32
FP16 = mybir.dt.float16
BF16 = mybir.dt.bfloat16
BF16_NP = ml_dtypes.bfloat16
AF = mybir.ActivationFunctionType

N_HEADS = 32
N_KV = 16
HEAD_DIM = 128
ROPE_THETA = 10000.0
SOFTCAP = 50.0
SCALE = 1.0 / 12.0  # 1/sqrt(144)
L = 2048
D = 4608
N_CORES = 8
QH = N_HEADS // N_CORES        # 4 local q heads
KVH = N_KV // N_CORES          # 2 local kv heads
KC = D // 128                  # 36 contraction chunks
NQ = L // 512                  # 4 l-chunks of 512
LT = L // 128                  # 16 l-tiles of 128
QUART = 9 * 512                # x/wq quarter width (9 k-chunks)


def _emit(nc):
    # DRAM tensors in SBUF-image layout (see make_in_maps)
    xq_d = nc.dram_tensor("xq", [NQ, 4, 128, QUART], BF16, kind="ExternalInput")
    wqq_d = nc.dram_tensor("wqq", [4, 128, QUART], BF16, kind="ExternalInput")
    wkh_d = nc.dram_tensor("wkh", [2, 128, 18 * 256], BF16, kind="ExternalInput")
    wvh_d = nc.dram_tensor("wvh", [2, 128, 18 * 256], BF16, kind="ExternalInput")
    wo4_d = nc.dram_tensor("wo4", [QH, 128, D], BF16, kind="ExternalInput")
    cost_d = nc.dram_tensor("cost", [128, L], BF16, kind="ExternalInput")
    sint_d = nc.dram_tensor("sint", [128, L], BF16, kind="ExternalInput")
    masks_d = nc.dram_tensor("masks", [128, 4 * 512], BF16, kind="ExternalInput")
    out_d = nc.dram_tensor("out", [L, D], BF16, kind="ExternalOutput")

    with tile.TileContext(nc) as tc:
        with (
            tc.tile_pool(name="const", bufs=1) as const,
            tc.tile_pool(name="persist", bufs=1) as persist,
            tc.tile_pool(name="xpb", bufs=1) as xpb,    # x chunks 1,3; chunk 3 outlives phase 1
            tc.tile_pool(name="wvp", bufs=1) as wvp,    # wv weights; outlive phase 1
        ):
            QT = [persist.tile([128, L], BF16, tag=f"qt{h}", name=f"qt{h}") for h in range(QH)]
            KT = [persist.tile([128, L], BF16, tag=f"kt{g}", name=f"kt{g}") for g in range(KVH)]
            # V extended with a ones column per k-tile: [128, 16*129]
            VE = [persist.tile([128, LT * 129], BF16, tag=f"ve{g}", name=f"ve{g}") for g in range(KVH)]

            ident = const.tile([128, 128], BF16)
            cost = const.tile([128, L], BF16)
            sint = const.tile([128, L], BF16)
            maskt = const.tile([128, 4 * 512], BF16)
            wvs = wvp.tile([128, KC * 256], BF16, name="wvs")

            deferred = _phase1(nc, tc, ident, cost, sint, maskt, wvs, QT, KT, VE,
                               xq_d, wqq_d, wkh_d, wvh_d, cost_d, sint_d,
                               masks_d, xpb)
            _phase2(nc, tc, ident, maskt, QT, KT, VE, wvs, wo4_d, out_d, deferred)
    return nc


def _phase1(nc, tc, ident, cost, sint, maskt, wvs, QT, KT, VE,
            xq_d, wqq_d, wkh_d, wvh_d, cost_d, sint_d, masks_d, xpb):
    with (
        tc.tile_pool(name="xpa", bufs=1) as xpa,
        tc.tile_pool(name="wqk", bufs=1) as wqk,
        tc.tile_pool(name="rtmp", bufs=3) as rtmp,
        tc.tile_pool(name="pj_psum", bufs=2, space="PSUM") as pj_psum,
        tc.tile_pool(name="vp_psum", bufs=2, space="PSUM") as vp_psum,
    ):
        wqs = wqk.tile([128, KC * 512], BF16, name="wqs")
        wks = wqk.tile([128, KC * 256], BF16, name="wks")

        def load_x(nq, dst):
            for j in range(4):
                nc.sync.dma_start(dst[:, j * QUART:(j + 1) * QUART], xq_d[nq, j])

        # interleave wq and x(0) quarters so the first Q matmuls start early
        x_first = xpa.tile([128, KC * 512], BF16, tag="xa", name="x0")
        for j in range(4):
            nc.sync.dma_start(wqs[:, j * QUART:(j + 1) * QUART], wqq_d[j])
            nc.sync.dma_start(x_first[:, j * QUART:(j + 1) * QUART], xq_d[0, j])
        nc.sync.dma_start(cost[:], cost_d[:])
        nc.sync.dma_start(sint[:], sint_d[:])
        for j in range(2):
            nc.sync.dma_start(wks[:, j * 18 * 256:(j + 1) * 18 * 256], wkh_d[j])
        for j in range(2):
            nc.sync.dma_start(wvs[:, j * 18 * 256:(j + 1) * 18 * 256], wvh_d[j])
        nc.sync.dma_start(maskt[:], masks_d[:])
        make_identity(nc, ident[:])

        def drain_rope(ps, dst, nq):
            """psum [128,512] f32 -> rope -> dst bf16 [128,512] slice."""
            cols = slice(nq * 512, (nq + 1) * 512)
            raw = rtmp.tile([128, 512], F32, tag="raw")
            nc.scalar.activation(raw[:], ps[:], AF.Copy)
            swap = rtmp.tile([128, 512], F32, tag="swap")
            nc.scalar.activation(swap[0:64, :], ps[64:128, :], AF.Copy)
            nc.scalar.activation(swap[64:128, :], ps[0:64, :], AF.Copy)
            nc.vector.tensor_mul(raw[:], raw[:], cost[:, cols])
            nc.vector.tensor_mul(swap[:], swap[:], sint[:, cols])
            nc.vector.tensor_add(dst[:, cols], raw[:], swap[:])

        def alloc_x(nq):
            pool = xpb if nq % 2 == 1 else xpa
            tag = "xb" if nq % 2 == 1 else "xa"
            return pool.tile([128, KC * 512], BF16, tag=tag, name=f"x{nq}")

        xc = x_first
        for nq in range(NQ):
            if nq + 1 < NQ:
                xnext = alloc_x(nq + 1)
                load_x(nq + 1, xnext)
            for h in range(QH):
                ps = pj_psum.tile([128, 512], F32, tag="qk")
                for k in range(KC):
                    nc.tensor.matmul(
                        ps[:], wqs[:, k * 512 + h * 128:k * 512 + (h + 1) * 128],
                        xc[:, k * 512:(k + 1) * 512],
                        start=(k == 0), stop=(k == KC - 1))
                drain_rope(ps, QT[h], nq)
            for g in range(KVH):
                ps = pj_psum.tile([128, 512], F32, tag="qk")
                for k in range(KC):
                    nc.tensor.matmul(
                        ps[:], wks[:, k * 256 + g * 128:k * 256 + (g + 1) * 128],
                        xc[:, k * 512:(k + 1) * 512],
                        start=(k == 0), stop=(k == KC - 1))
                drain_rope(ps, KT[g], nq)
            if nq == NQ - 1:
                return [(nq, sub, xc) for sub in range(4)]  # V deferred to phase 2
            for sub in range(4):
                mk = nq * 4 + sub
                ps = vp_psum.tile([128, KVH * 128], F32, tag="vps")
                for k in range(KC):
                    nc.tensor.matmul(
                        ps[:], xc[:, k * 512 + sub * 128:k * 512 + (sub + 1) * 128],
                        wvs[:, k * 256:(k + 1) * 256],
                        start=(k == 0), stop=(k == KC - 1))
                for g in range(KVH):
                    nc.vector.tensor_copy(
                        VE[g][:, mk * 129:mk * 129 + 128],
                        ps[:, g * 128:(g + 1) * 128])
                    nc.gpsimd.memset(VE[g][:, mk * 129 + 128:mk * 129 + 129], 1.0)
            xc = xnext


def _phase2(nc, tc, ident, maskt, QT, KT, VE, wvs, wo4_d, out_d, deferred_v):
    with (
        tc.tile_pool(name="wo", bufs=1) as wop,
        tc.tile_pool(name="pt", bufs=2) as ptp,
        tc.tile_pool(name="tt", bufs=2) as ttp,
        tc.tile_pool(name="attnt", bufs=3) as attp,
        tc.tile_pool(name="small", bufs=2) as small,
        tc.tile_pool(name="ostage", bufs=2) as ostage,
        tc.tile_pool(name="sc_psum", bufs=2, space="PSUM") as sc_psum,
        tc.tile_pool(name="pv_psum", bufs=2, space="PSUM") as pv_psum,
        tc.tile_pool(name="op_psum", bufs=3, space="PSUM") as op_psum,
        tc.tile_pool(name="atr_psum", bufs=1, space="PSUM") as atr_psum,
    ):
        WO = wop.tile([128, QH * D], BF16, name="wos")
        for h in range(QH):
            nc.sync.dma_start(WO[:, h * D:(h + 1) * D], wo4_d[h])

        groups = [(nq, h) for nq in range(NQ) for h in range(QH)]
        state = {}
        att_of = {nq: [None] * QH for nq in range(NQ)}

        # Two filler queues of (group_tag, pe_cost_ns, emit_fn):
        #   qA: exp-dependent units (PV / transpose) - eligible 2 pairs into
        #       the group after theirs.
        #   qB: exp-independent units (o_proj, deferred V) - eligible once qA
        #       holds nothing at or before their tag (preserves the
        #       T(nq,3) -> o_proj(nq) emission-order dependency).
        qA = deque()
        qB = deque()

        def emit_next(i, pk):
            """Emit one eligible filler unit; return its PE cost or None."""
            if qA and (qA[0][0] <= i - 2 or (qA[0][0] == i - 1 and pk >= 2)):
                g, cost, fn = qA.popleft()
                fn()
                return cost
            if qB and (not qA or qA[0][0] > qB[0][0]):
                g, cost, fn = qB.popleft()
                fn()
                return cost
            return None

        def pump(i, pk, ns):
            while ns > 0:
                c = emit_next(i, pk)
                if c is None:
                    return
                ns -= c

        def pump_guard(i):
            # bound pipeline lag: everything enqueued at group <= i-2 must be
            # emitted before group i starts reusing its tile buffers
            while qA and qA[0][0] <= i - 2:
                qA.popleft()[2]()
            while qB and qB[0][0] <= i - 2:
                qB.popleft()[2]()

        def make_v_unit(vnq, sub, xc):
            def emit():
                mk = vnq * 4 + sub
                ps = op_psum.tile([128, 512], F32, tag="op")
                for k in range(KC):
                    nc.tensor.matmul(
                        ps[:, 0:KVH * 128],
                        xc[:, k * 512 + sub * 128:k * 512 + (sub + 1) * 128],
                        wvs[:, k * 256:(k + 1) * 256],
                        start=(k == 0), stop=(k == KC - 1))
                for g in range(KVH):
                    nc.vector.tensor_copy(
                        VE[g][:, mk * 129:mk * 129 + 128],
                        ps[:, g * 128:(g + 1) * 128])
                    nc.gpsimd.memset(VE[g][:, mk * 129 + 128:mk * 129 + 129], 1.0)
            return (-10, 3900, emit)

        for u in deferred_v:
            qB.append(make_v_unit(*u))

        def emit_scores_pair(nq, h, pk, tt, pt):
            """two score tiles -> f32 psum each -> trimmed tanh into the fp16
            tt quad staging; exp runs once per completed [128,2048] quad."""
            g = h // 2
            for half in range(2):
                mk = 2 * pk + half
                o = mk - 4 * nq
                c0 = max(0, o) * 128
                slot = (mk % 4) * 512
                ps = sc_psum.tile([128, 512], F32, tag="sc")
                nc.tensor.matmul(
                    ps[:, c0:512],
                    KT[g][:, mk * 128:(mk + 1) * 128],
                    QT[h][:, nq * 512 + c0:(nq + 1) * 512])
                nc.scalar.activation(
                    tt[:, slot + c0:slot + 512], ps[:, c0:512],
                    AF.Tanh, scale=SCALE / SOFTCAP)
                if mk % 4 == 3:
                    c = mk // 4
                    nc.scalar.activation(
                        pt[:, c * 2048:(c + 1) * 2048], tt[:], AF.Exp,
                        scale=SOFTCAP)

        def emit_masks(nq, h, pt):
            for o in range(4):
                mk = 4 * nq + o
                c0 = o * 128
                sl = slice(mk * 512 + c0, mk * 512 + 512)
                nc.gpsimd.tensor_mul(
                    pt[:, sl], pt[:, sl], maskt[:, o * 512 + c0:(o + 1) * 512])

        def make_pv_unit(idx, nq, h, s):
            def emit():
                st = state[(nq, h)]
                g = h // 2
                nks = 4 * nq + s + 1
                pv = pv_psum.tile([128, 129], F32, tag="pv")
                for mk in range(nks):
                    nc.tensor.matmul(
                        pv[:],
                        st["pt"][:, mk * 512 + s * 128:mk * 512 + (s + 1) * 128],
                        VE[g][:, mk * 129:(mk + 1) * 129],
                        start=(mk == 0), stop=(mk == nks - 1))
                recip = small.tile([128, 1], F32, tag="recip")
                nc.vector.reciprocal(recip[:], pv[:, 128:129])
                aq = small.tile([128, 128], BF16, tag=f"attnq{s}")
                nc.vector.tensor_scalar_mul(aq[:], pv[:, 0:128], recip[:])
                st["attnq"][s] = aq
            return (idx, 200 + (4 * nq + s + 1) * 110, emit)

        def make_tr_unit(idx, nq, h, s):
            def emit():
                st = state[(nq, h)]
                tp = atr_psum.tile([128, 128], BF16, tag="atr")
                nc.tensor.transpose(tp[:], st["attnq"][s][:], ident[:])
                nc.vector.tensor_copy(st["attnT"][:, s * 128:(s + 1) * 128], tp[:])
            return (idx, 180, emit)

        def make_oproj_unit(idx, nq, s, jg, ob):
            def emit():
                for j in range(3 * jg, 3 * jg + 3):
                    po = op_psum.tile([128, 512], F32, tag="op")
                    for h in range(QH):
                        nc.tensor.matmul(
                            po[:], att_of[nq][h][:, s * 128:(s + 1) * 128],
                            WO[:, h * D + j * 512:h * D + (j + 1) * 512],
                            start=(h == 0), stop=(h == QH - 1))
                    nc.vector.tensor_copy(
                        ob[:, (j - 3 * jg) * 512:(j - 3 * jg + 1) * 512], po[:])
                row = nq * 512 + s * 128
                nc.sync.dma_start(
                    out_d[row:row + 128, jg * 1536:(jg + 1) * 1536], ob[:])
            return (idx, 3100, emit)

        for i, (nq, h) in enumerate(groups):
            npairs = 2 * nq + 2
            attnT = attp.tile([128, 512], BF16, tag=f"at{h}", name=f"at{h}")
            att_of[nq][h] = attnT
            pt = ptp.tile([128, LT * 512], BF16, tag="pt", name="pt")
            st = {"pt": pt, "attnq": [None] * 4, "attnT": attnT}
            state[(nq, h)] = st
            tt = None
            for pk in range(npairs):
                pump_guard(i)
                if pk % 2 == 0:
                    tt = ttp.tile([128, 2048], FP16, tag="tt")
                emit_scores_pair(nq, h, pk, tt, pt)
                pump(i, pk, 2200)
            emit_masks(nq, h, pt)
            for s in range(4):
                qA.append(make_pv_unit(i, nq, h, s))
                qA.append(make_tr_unit(i, nq, h, s))
            if h == QH - 1:
                for s in range(4):
                    for jg in range(3):
                        ob = ostage.tile([128, 1536], BF16, tag="ob")
                        qB.append(make_oproj_unit(i, nq, s, jg, ob))
        while qA:
            qA.popleft()[2]()
        while qB:
            qB.popleft()[2]()


_CACHED_NC = {}


def build():
    if "nc" not in _CACHED_NC:
        nc = bacc.Bacc("TRN2", target_bir_lowering=False, debug=False)
        _emit(nc)
        nc.compile()
        _CACHED_NC["nc"] = nc
    return _CACHED_NC["nc"]


def host_tables():
    inv_freq = 1.0 / (ROPE_THETA ** (np.arange(0, HEAD_DIM, 2, dtype=np.float32) / HEAD_DIM))
    ang = np.arange(L, dtype=np.float32)[:, None] * inv_freq[None, :]  # [L, 64]
    cos, sin = np.cos(ang), np.sin(ang)
    cosT = np.concatenate([cos.T, cos.T], axis=0).astype(BF16_NP)
    sinT = np.concatenate([-sin.T, sin.T], axis=0).astype(BF16_NP)
    return np.ascontiguousarray(cosT), np.ascontiguousarray(sinT)


def host_masks():
    k = np.arange(128)[:, None]
    q = np.arange(512)[None, :]
    m = np.concatenate([(q >= k + 128 * o) for o in range(4)], axis=1).astype(BF16_NP)
    return np.ascontiguousarray(m)  # [128, 2048]


def make_in_maps(x, wq, wk, wv, wo):
    cosT, sinT = host_tables()
    masks = host_masks()
    xt = x.reshape(L, D).T.astype(BF16_NP)  # [D, L]
    # x image: [nq, j, p, k9*512+c] = xt[(9j+k9)*128+p, nq*512+c]
    xi = xt.reshape(KC, 128, NQ, 512).transpose(2, 0, 1, 3)  # [NQ, KC, 128, 512]
    xi = xi.reshape(NQ, 4, 9, 128, 512).transpose(0, 1, 3, 2, 4)
    xi = np.ascontiguousarray(xi.reshape(NQ, 4, 128, QUART))
    in_maps = []
    for c in range(N_CORES):
        qs = slice(c * QH * 128, (c + 1) * QH * 128)
        kvs = slice(c * KVH * 128, (c + 1) * KVH * 128)
        wqt = wq[qs].T.astype(BF16_NP)   # [D, 512]
        wkt = wk[kvs].T.astype(BF16_NP)  # [D, 256]
        wvt = wv[kvs].T.astype(BF16_NP)
        wot = wo[:, qs].T.astype(BF16_NP)  # [512, D]
        wqi = wqt.reshape(KC, 128, 512).transpose(1, 0, 2)
        wqi = np.ascontiguousarray(
            wqi.reshape(128, 4, 9, 512).transpose(1, 0, 2, 3).reshape(4, 128, QUART))
        wki = wkt.reshape(KC, 128, 256).transpose(1, 0, 2)
        wki = np.ascontiguousarray(
            wki.reshape(128, 2, 18, 256).transpose(1, 0, 2, 3).reshape(2, 128, 18 * 256))
        wvi = wvt.reshape(KC, 128, 256).transpose(1, 0, 2)
        wvi = np.ascontiguousarray(
            wvi.reshape(128, 2, 18, 256).transpose(1, 0, 2, 3).reshape(2, 128, 18 * 256))
        wo4 = np.ascontiguousarray(wot.reshape(QH, 128, D))
        in_maps.append({
            "xq": xi,
            "wqq": wqi,
            "wkh": wki,
            "wvh": wvi,
            "wo4": wo4,
            "cost": cosT,
            "sint": sinT,
            "masks": masks,
        })
    return in_maps


def run(inputs, trace=False, trace_kwargs=None):
    from concourse.bass_utils import run_bass_kernel_spmd

    nc = build()
    x = np.asarray(inputs["x"], dtype=np.float32)
    in_maps = make_in_maps(
        x,
        np.asarray(inputs["wq"], dtype=np.float32),
        np.asarray(inputs["wk"], dtype=np.float32),
        np.asarray(inputs["wv"], dtype=np.float32),
        np.asarray(inputs["wo"], dtype=np.float32),
    )
    res = run_bass_kernel_spmd(
        nc, in_maps, core_ids=list(range(N_CORES)),
        trace=trace, **(trace_kwargs or {}))
    out = np.zeros((L, D), dtype=np.float32)
    for c in range(N_CORES):
        out += res.results[c]["out"].astype(np.float32)
    return out.reshape(x.shape), res


def kernel(**inputs) -> np.ndarray:
    out, _ = run(inputs, trace=False)
    return out
